# revision 26
# baseline (speedup 1.0000x reference)
"""Sparse (Cantor-coordinate k-NN) attention on 8 Trainium2 NeuronCores.

Strategy
--------
Shard (B=2) x (H=8) head-pairs across 8 cores: core c handles batch c//4,
heads 2*(c%4) and 2*(c%4)+1.

The k-NN routing in 1-D Cantor-coordinate space selects, for each query, a
contiguous window of 128 keys in *sorted-coordinate order*.  So instead of a
[S,K,HD] gather we permute tokens into sorted order on the host (pure data
movement) and run *banded* dense attention on the device: each 128-query
block attends to a 128-aligned key range of width R (384 for the reference
data), with an additive -1e30 mask encoding each query's exact 128-key
window.  Masks/permutation depend only on the shared cantor_coords and are
computed once on the host and replicated (see sharding hint).

Device kernel per core:
  x_sorted -> PE-transpose -> x^T ; QKV projection with fp32r matmuls
  (q^T,k^T,v^T head-transposed; v^T then PE-transposed to natural) ;
  per query block: scores = q^T.T @ k^T slab (fp32r), mask-add (DVE),
  exp+rowsum (ACT accum), probs transpose (PE, packed evictions),
  AV accumulate (PE), normalize on PSUM eviction (DVE) ; out^T assembly ;
  out-proj partial (fp32r, bias folded in via a rank-1 matmul) ;
  ReduceScatter over the 4 cores of the same batch, interleaved per
  512-token chunk ; host reassembles + inverse-permutes.
"""

import numpy as np

S = 2048
D = 512
HEADS = 8
HD = 64
KNN = 128
QB = 128          # queries per block
NBLK = S // QB
SCALE = 1.0 / (HD ** 0.5)
NEG = -1.0e30
F32 = np.float32


# ----------------------------------------------------------------- routing --
def _routing(coords):
    """Sorted order + per-query window starts (exact top-k set in 1-D)."""
    order = np.argsort(coords, kind="stable")
    cs32 = coords[order]  # compare in f32 exactly as the reference does
    w = np.zeros(S, np.int64)
    l = 0
    for p in range(S):
        lo = max(0, p - KNN + 1)
        hi = min(p, S - KNN)
        l = min(max(l, lo), hi)
        while l < hi and (cs32[p] - cs32[l]) > (cs32[l + KNN] - cs32[p]):
            l += 1
        w[p] = l
    return order, w


def _blocks(w):
    """128-aligned key base per query block and the uniform slab width R."""
    kb = np.zeros(NBLK, np.int64)
    ends = np.zeros(NBLK, np.int64)
    for b in range(NBLK):
        ws = w[b * QB:(b + 1) * QB]
        kb[b] = (ws.min() // 128) * 128
        ends[b] = -((-(ws.max() + KNN)) // 128) * 128
    R = int((ends - kb).max())
    R = max(R, 256)
    kb = np.minimum(kb, S - R)
    return kb, R


def _build_mask(w, kb, R):
    """Additive mask [S, R]: 0 inside the query's 128-key window, -1e30 out."""
    mask = np.full((S, R), NEG, dtype=F32)
    j = np.arange(R)
    for b in range(NBLK):
        rows = slice(b * QB, (b + 1) * QB)
        rel_lo = (w[rows] - kb[b])[:, None]          # [QB,1]
        inside = (j[None, :] >= rel_lo) & (j[None, :] < rel_lo + KNN)
        blockview = mask[rows]
        blockview[inside] = 0.0
    return mask


# ------------------------------------------------------------ bass program --
def _build_nc(R, kb):
    import concourse.bass as bass
    import concourse.mybir as mybir
    from concourse import bacc
    from concourse.tile import TileContext
    from concourse.masks import make_identity

    f32 = mybir.dt.float32
    f32r = mybir.dt.float32r
    RC = R // 128  # key chunks per slab

    def r_(ap):
        return ap.bitcast(f32r)

    nc = bacc.Bacc(num_devices=8)
    xs = nc.declare_dram_parameter("xs", [S, D], f32, isOutput=False)
    wqkv = nc.declare_dram_parameter("wqkv", [D, 384], f32, isOutput=False)
    bqkv = nc.declare_dram_parameter("bqkv", [384], f32, isOutput=False)
    wout = nc.declare_dram_parameter("wout", [128, D], f32, isOutput=False)
    boutq = nc.declare_dram_parameter("boutq", [D], f32, isOutput=False)
    maskp = nc.declare_dram_parameter("mask", [S, R], f32, isOutput=False)
    y = nc.declare_dram_parameter("y", [4, 128, D], f32, isOutput=True)

    with TileContext(nc) as tc:
        with (
            tc.tile_pool(name="consts", bufs=1) as consts,
            tc.tile_pool(name="big", bufs=1) as big,
            tc.tile_pool(name="xload", bufs=3) as xload,
            tc.tile_pool(name="work", bufs=3) as work,
            tc.tile_pool(name="probs", bufs=3) as probsp,
            tc.tile_pool(name="small", bufs=6) as small,
            tc.tile_pool(name="yout", bufs=3) as yout,
            tc.tile_pool(name="ps512", bufs=2, space="PSUM") as ps512,
            tc.tile_pool(name="pstr", bufs=2, space="PSUM") as pstr,
            tc.tile_pool(name="pssc", bufs=2, space="PSUM") as pssc,
            tc.tile_pool(name="psot", bufs=1, space="PSUM") as psot,
            tc.tile_pool(name="psav", bufs=1, space="PSUM") as psav,
            tc.tile_pool(name="dram", bufs=8, space="DRAM") as dram,
        ):
            ident = consts.tile([128, 128], f32)
            make_identity(nc, ident)
            ones_f = consts.tile([1, 128], f32)
            nc.vector.memset(ones_f, 1.0)
            ones_row = consts.tile([1, 128], f32r)
            nc.vector.tensor_copy(out=ones_row, in_=ones_f)

            wqkv_f = consts.tile([128, 4, 384], f32)
            nc.sync.dma_start(
                out=wqkv_f, in_=wqkv.rearrange("(kc p) n -> p kc n", p=128))
            wqkv_sb = consts.tile([128, 4, 384], f32r)
            nc.vector.tensor_copy(out=wqkv_sb[:, :, :], in_=wqkv_f[:, :, :])
            wout_f = consts.tile([128, D], f32)
            nc.sync.dma_start(out=wout_f, in_=wout[:, :])
            wout_sb = consts.tile([128, D], f32r)
            nc.vector.tensor_copy(out=wout_sb[:, :], in_=wout_f[:, :])
            bq_sb = consts.tile([128, 3], f32)
            nc.sync.dma_start(
                out=bq_sb, in_=bqkv.rearrange("(t p) -> p t", p=128))
            bo_f = consts.tile([1, D], f32)
            nc.sync.dma_start(
                out=bo_f,
                in_=bass.AP(tensor=boutq, offset=0, ap=[[0, 1], [1, D]]))
            bo_row = consts.tile([1, D], f32r)
            nc.vector.tensor_copy(out=bo_row, in_=bo_f)
            mask_sb = big.tile([128, NBLK, R], f32)
            nc.sync.dma_start(
                out=mask_sb, in_=maskp.rearrange("(b p) r -> p b r", p=128))

            xT = big.tile([128, 4, S], f32r)       # x^T, 4 chunks of D
            qT = big.tile([128, S], f32r)          # both heads stacked
            kT = big.tile([128, S], f32r)
            vT = big.tile([128, S], f32)
            vn = big.tile([128, S // 128, 130], f32)   # v nat + ones cols
            outT = big.tile([128, S], f32r)        # attn out^T (2 heads)

            # ---- phase 1: load x (sorted order) and transpose to x^T ----
            # 4 PE transposes pack into one PSUM bank -> single copy out
            for tt in range(S // 128):
                xt = xload.tile([128, D], f32, tag="xt")
                nc.sync.dma_start(out=xt, in_=xs[tt * 128:(tt + 1) * 128, :])
                tp = pstr.tile([128, 4, 128], f32, tag="tr")
                for kc in range(4):
                    nc.tensor.transpose(tp[:, kc, :],
                                        xt[:, kc * 128:(kc + 1) * 128], ident)
                if tt % 2 == 0:
                    nc.vector.tensor_copy(
                        out=xT[:, :, tt * 128:(tt + 1) * 128], in_=tp[:, :, :])
                else:
                    nc.scalar.copy(
                        out=xT[:, :, tt * 128:(tt + 1) * 128], in_=tp[:, :, :])

            # ---- phase 2: QKV projection (fp32r), all head-transposed ----
            for t, dest in ((0, qT), (1, kT), (2, vT)):
                for f in range(4):
                    ps = ps512.tile([128, 512], f32, tag="mm512")
                    for kc in range(4):
                        nc.tensor.matmul(
                            ps,
                            lhsT=wqkv_sb[:, kc, t * 128:(t + 1) * 128],
                            rhs=xT[:, kc, f * 512:(f + 1) * 512],
                            start=(kc == 0), stop=(kc == 3))
                    nc.vector.tensor_scalar_add(
                        out=dest[:, f * 512:(f + 1) * 512], in0=ps,
                        scalar1=bq_sb[:, t:t + 1])
            # v^T -> v natural via PE transpose (both heads in one shot)
            # layout per tile: [v_h0 (64) | ones | v_h1 (64) | ones] = 130
            # the ones column turns the AV matmul into AV+rowsum (col 64)
            for tt in range(S // 128):
                tp = pstr.tile([128, 128], f32, tag="tr")
                nc.tensor.transpose(tp, vT[:, tt * 128:(tt + 1) * 128], ident)
                dst = vn[:, tt, :]
                seg = bass.AP(tensor=dst.tensor, offset=dst.offset,
                              ap=[dst.ap[0], [65, 2], [1, 64]])
                if tt % 2 == 0:
                    nc.vector.tensor_copy(out=seg, in_=tp.rearrange(
                        "p (s c) -> p s c", s=2))
                else:
                    nc.scalar.copy(out=seg, in_=tp.rearrange(
                        "p (s c) -> p s c", s=2))
            nc.vector.memset(vn[:, :, 64:65], 1.0)
            nc.vector.memset(vn[:, :, 129:130], 1.0)

            # ---- phase 3: software-pipelined banded attention ----
            # unit u = (blk, h); stages lagged so the PE stream never stalls:
            #   S(u): scores matmul        -> psum
            #   ME(u): mask-add (DVE) + exp (ACT)
            #   T(u): probs transposes     -> psum -> copy to SBUF
            #   A(u): AV matmuls (+rowsum col), normalize, out^T transpose
            # plus out-proj + ReduceScatter per 4-block chunk.
            groups = [[0, 1, 2, 3], [4, 5, 6, 7]]
            ypart = dram.tile([4, 512, D], f32)
            U = NBLK * 2
            st = {}
            ot_tiles = {}

            def stage_S(u):
                blk, h = u // 2, u % 2
                base = int(kb[blk])
                hs = slice(h * 64, (h + 1) * 64)
                sc = pssc.tile([128, R], f32, tag="sc")
                nc.tensor.matmul(
                    sc, lhsT=qT[hs, blk * 128:(blk + 1) * 128],
                    rhs=kT[hs, base:base + R], start=True, stop=True)
                masked = work.tile([128, R], f32, tag="masked")
                nc.vector.tensor_add(out=masked, in0=sc,
                                     in1=mask_sb[:, blk, :])
                probs = work.tile([128, R], f32, tag="probs")
                nc.scalar.activation(
                    out=probs, in_=masked,
                    func=mybir.ActivationFunctionType.Exp,
                    scale=float(SCALE))
                st[u] = {"probs": probs}

            def stage_T(u):
                probs = st[u]["probs"]
                tp = pstr.tile([128, RC, 128], f32, tag="tr")
                for ck in range(RC):
                    nc.tensor.transpose(
                        tp[:, ck, :], probs[:, ck * 128:(ck + 1) * 128], ident)
                ptsb = probsp.tile([128, RC, 128], f32, tag="ptsb")
                if u % 2 == 0:
                    nc.vector.tensor_copy(out=ptsb[:, :, :], in_=tp[:, :, :])
                else:
                    nc.scalar.copy(out=ptsb[:, :, :], in_=tp[:, :, :])
                st[u]["ptsb"] = ptsb

            def stage_A(u):
                blk, h = u // 2, u % 2
                base = int(kb[blk])
                ptsb = st[u]["ptsb"]
                av = psav.tile([128, 65], f32, tag="av")
                for ck in range(RC):
                    nc.tensor.matmul(
                        av, lhsT=ptsb[:, ck, :],
                        rhs=vn[:, base // 128 + ck,
                               h * 65:h * 65 + 65],
                        start=(ck == 0), stop=(ck == RC - 1))
                recip = small.tile([128, 1], f32, tag="recip")
                nc.vector.reciprocal(out=recip, in_=av[:, 64:65])
                outq = work.tile([128, 64], f32, tag="outq")
                nc.vector.tensor_scalar_mul(out=outq, in0=av[:, 0:64],
                                            scalar1=recip)
                if h == 0:
                    ot = psot.tile([64, 256], f32, tag="ot")
                    ot_tiles[blk] = ot
                ot = ot_tiles[blk]
                nc.tensor.transpose(ot[:, h * 128:(h + 1) * 128], outq, ident)
                if h == 1:
                    nc.vector.tensor_copy(
                        out=outT[0:64, blk * 128:(blk + 1) * 128],
                        in_=ot[:, 0:128])
                    nc.scalar.copy(
                        out=outT[64:128, blk * 128:(blk + 1) * 128],
                        in_=ot[:, 128:256])
                    del ot_tiles[blk]
                    # out-proj for this token chunk (bias via rank-1 matmul)
                    ps = ps512.tile([128, 512], f32, tag="mm512")
                    nc.tensor.matmul(
                        ps, lhsT=outT[:, blk * 128:(blk + 1) * 128],
                        rhs=wout_sb, start=True, stop=False)
                    nc.tensor.matmul(ps, lhsT=ones_row, rhs=bo_row,
                                     start=False, stop=True)
                    ysb = yout.tile([128, D], f32, tag="ysb")
                    if blk % 2 == 0:
                        nc.vector.tensor_copy(out=ysb, in_=ps)
                    else:
                        nc.scalar.copy(out=ysb, in_=ps)
                    nc.sync.dma_start(
                        out=ypart[blk // 4,
                                  (blk % 4) * 128:(blk % 4 + 1) * 128, :],
                        in_=ysb)
                    if blk % 4 == 3:
                        q = blk // 4
                        rs_out = dram.tile([128, D], f32, tag="rs_out")
                        nc.gpsimd.collective_compute(
                            "ReduceScatter", mybir.AluOpType.add,
                            replica_groups=groups,
                            ins=[ypart[q].opt()], outs=[rs_out.opt()])
                        nc.sync.dma_start(out=y[q], in_=rs_out)
                del st[u]

            for t in range(U + 2):
                if t < U:
                    stage_S(t)
                if 0 <= t - 1 < U:
                    stage_T(t - 1)
                if 0 <= t - 2 < U:
                    stage_A(t - 2)
    nc.finalize()
    return nc


_CACHE = {}
_LAST_NC = None
_LAST_IN_MAPS = None


def _get_nc(R, kb):
    key = (R, tuple(int(v) for v in kb))
    if key not in _CACHE:
        _CACHE[key] = _build_nc(R, kb)
    return _CACHE[key]


# ---------------------------------------------------------------- kernel ----
def kernel(x, cantor_coords, W_qkv, b_qkv, W_out, b_out):
    from concourse.bass_utils import run_bass_kernel_spmd

    x = np.ascontiguousarray(x, dtype=F32)
    coords = np.ascontiguousarray(cantor_coords, dtype=F32)
    W_qkv = np.ascontiguousarray(W_qkv, dtype=F32)
    b_qkv = np.ascontiguousarray(b_qkv, dtype=F32)
    W_out = np.ascontiguousarray(W_out, dtype=F32)
    b_out = np.ascontiguousarray(b_out, dtype=F32)
    B = x.shape[0]
    assert x.shape == (B, S, D) and coords.shape == (S,)

    order, w = _routing(coords)
    kb, R = _blocks(w)
    assert R <= 512 and R % 128 == 0, R
    mask = _build_mask(w, kb, R)
    nc = _get_nc(R, kb)

    in_maps = []
    for c in range(8):
        b = c // 4
        h0 = 2 * (c % 4)
        cols = slice(h0 * 64, (h0 + 2) * 64)
        wq = W_qkv[:, cols]
        wk = W_qkv[:, 512 + h0 * 64: 512 + (h0 + 2) * 64]
        wv = W_qkv[:, 1024 + h0 * 64: 1024 + (h0 + 2) * 64]
        in_maps.append({
            "xs": np.ascontiguousarray(x[b][order]),
            "wqkv": np.ascontiguousarray(
                np.concatenate([wq, wk, wv], axis=1)),
            "bqkv": np.ascontiguousarray(np.concatenate(
                [b_qkv[cols], b_qkv[512 + h0 * 64: 512 + (h0 + 2) * 64],
                 b_qkv[1024 + h0 * 64: 1024 + (h0 + 2) * 64]])),
            "wout": np.ascontiguousarray(W_out[h0 * 64:(h0 + 2) * 64, :]),
            "boutq": np.ascontiguousarray(b_out * 0.25),
            "mask": mask,
        })

    nc_obj = nc
    global _LAST_NC, _LAST_IN_MAPS
    _LAST_NC, _LAST_IN_MAPS = nc_obj, in_maps
    res = run_bass_kernel_spmd(nc_obj, in_maps, list(range(8))).results

    out = np.empty((B, S, D), dtype=F32)
    ys = np.empty((S, D), dtype=F32)
    for b in range(B):
        for r in range(4):
            yr = res[4 * b + r]["y"]          # [4, 128, D]
            for q in range(4):
                ys[q * 512 + r * 128: q * 512 + (r + 1) * 128] = yr[q]
        out[b][order] = ys
    return out
